# revision 15
# baseline (speedup 1.0000x reference)
"""Causal self-attention (B=8, T=1024, C=768, H=8 heads) for 8 TRN2 NeuronCores.

Strategy: pure data parallelism - one batch element per core, weights
replicated, no collectives.

v2 pipeline (PE matmul operands in bf16 -> 1 cycle/row at any N, dodging the
fp32r 4x penalty for N<256; PSUM accumulation stays fp32):
  1. x -> x^T via PE transposes (fp32r, 1.5 c/r), PSUM->SBUF copyback casts to
     bf16. Weights DMA'd fp32 on the Scalar-engine HWDGE queue (separate from
     the x/repack/scatter traffic on the Sync queue) and cast to bf16 on the
     idle ACT engine through small rotating staging tiles.
  2. v = x @ W_v + b_v token-major, packed per 128-token block as
     v_aug [128, 8*97] bf16: per head 96 value columns plus a ones column (the
     ones column makes the P@V matmul also produce the softmax denominator).
  3. q/k projection computed DENSE (12 output blocks of 128 features, M=128,
     25% less PE time than 96-wide per-head blocks); copyback adds bias and
     casts to bf16; per-head [96, 1024] q^T/k^T tiles are then sliced out by
     SBUF->SBUF DMA (row shift, off-engine).
  4. Per head: S^T[tk,q] = k^T.T @ q^T per 128-key block over the causally
     valid query range; P = exp(S*scale) via ACT (bf16 out); diagonal triangle
     masked by a DVE bf16 multiply; y_aug^T = sum_tk v_aug^T P accumulated in
     PSUM (row 96 = denominator). Dense-projection half-blocks (and later
     out-proj blocks) are emitted as FILLER between S key-blocks so the PE
     works through the ~6us/head ACT exp latency instead of stalling on PSUM
     slot reuse.
  5. Softmax tails without ACT Ln/Exp round-trips: denominator rows are
     DMA-gathered onto separate partitions of one [8,1024] tile, one DVE
     reciprocal per batch of heads (cost ~ columns, partitions free), then a
     rank-1 PE matmul (ones[1,96] (x) rc[1,1024]) broadcasts the reciprocal
     across partitions into PSUM; y_n = y * bc on DVE -> scatter-DMA into
     feature-packed yT tiles.
  6. out = y @ W_proj + b_proj split into G1 (feature blocks cb0..4 = heads
     0..5, runs as filler inside heads 6/7's S) accumulating into SBUF o_acc
     with bias, and G2 (cb5 = heads 6/7) after the last tail + final DVE add
     -> DMA out on the Scalar queue.

PSUM: tag "big" [128,1024] bufs=3 (6 banks) for s/v/o psum, tag "bank1"
[*,512] bufs=2 (2 banks) for transpose groups, dense qk half-blocks and the
P@V y accumulators.
"""
import sys

sys.path.insert(0, "/opt/trn_rl_repo")

import numpy as np

T, C, H, D = 1024, 768, 8, 96
C3 = 3 * C
C2 = 2 * C
P = 128
NT = T // P   # 8 token blocks
NCB = C // P  # 6 feature blocks
DA = D + 1    # 97: head dim + denominator column
NQK = 12      # dense q/k projection output blocks (6 q + 6 k)

_CACHE = {}


def _build():
    import concourse.bacc as bacc
    import concourse.mybir as mybir
    import concourse.tile as tile
    from concourse.masks import make_identity

    F32 = mybir.dt.float32
    F32R = mybir.dt.float32r
    BF16 = mybir.dt.bfloat16
    Exp = mybir.ActivationFunctionType.Exp
    Copy = mybir.ActivationFunctionType.Copy
    is_ge = mybir.AluOpType.is_ge
    SCALE = 1.0 / float(np.sqrt(D))

    nc = bacc.Bacc("TRN2", target_bir_lowering=False, debug=False, num_devices=8)
    x_d = nc.dram_tensor("x", [T, C], F32, kind="ExternalInput").ap()
    wa_d = nc.dram_tensor("W_attn", [C, C3], F32, kind="ExternalInput").ap()
    ba_d = nc.dram_tensor("b_attn", [C3], F32, kind="ExternalInput").ap()
    wp_d = nc.dram_tensor("W_proj", [C, C], F32, kind="ExternalInput").ap()
    bp_d = nc.dram_tensor("b_proj", [C], F32, kind="ExternalInput").ap()
    out_d = nc.dram_tensor("out", [T, C], F32, kind="ExternalOutput").ap()

    # head h's 96 q (or k) rows live in dense block m1 rows [r1, r1+len1) and
    # (if spilling) block m1+1 rows [0, 96-len1)
    def head_segs(h):
        g0 = D * h
        m1, r1 = g0 // P, g0 % P
        len1 = min(P - r1, D)
        segs = [(m1, r1, 0, len1)]
        if len1 < D:
            segs.append((m1 + 1, 0, len1, D - len1))
        return segs

    with tile.TileContext(nc) as tc:
        with tc.tile_pool(name="const", bufs=1) as const_p, \
             tc.tile_pool(name="vp", bufs=1) as v_p, \
             tc.tile_pool(name="yt", bufs=1) as yT_p, \
             tc.tile_pool(name="ynp", bufs=4) as yn_p, \
             tc.tile_pool(name="rcr", bufs=2) as rcr_p, \
             tc.tile_pool(name="ysb", bufs=12) as ysb_p, \
             tc.tile_pool(name="pt", bufs=10) as p_p, \
             tc.tile_pool(name="qkh", bufs=8) as qkh_p, \
             tc.tile_pool(name="qkd", bufs=6) as qkd_p, \
             tc.tile_pool(name="wpb", bufs=1) as wpB_p, \
             tc.tile_pool(name="ps", bufs=1, space="PSUM") as ps:

            # ---------------- constants ----------------
            identS = const_p.tile([P, P], F32, name="identS")
            make_identity(nc, identS)
            identF = const_p.tile([P, P], F32R, name="identF")
            nc.vector.tensor_copy(identF[:], identS[:])
            identR = identF
            triB = const_p.tile([P, P], BF16, name="triB")
            # lower-left triangle: triB[tk, u] = 1.0 iff u >= tk
            nc.gpsimd.memset(triB, 1.0)
            nc.gpsimd.affine_select(
                out=triB, in_=triB, compare_op=is_ge, fill=0.0,
                base=0, pattern=[[1, P]], channel_multiplier=-1)
            onesMS = const_p.tile([1, D], F32, name="onesMS")
            nc.vector.memset(onesMS, 1.0)
            onesMR = const_p.tile([1, D], F32R, name="onesMR")
            nc.vector.tensor_copy(onesMR[:], onesMS[:])
            onesHB = const_p.tile([P, H], BF16, name="onesHB")
            nc.vector.memset(onesHB, 1.0)
            bqk = const_p.tile([P, NQK], F32, name="bqk")
            bvb = const_p.tile([P, C], F32, name="bvb")
            bpb = const_p.tile([P, C], F32, name="bpb")
            den8 = const_p.tile([H, T], BF16, name="den8")
            rc8 = const_p.tile([H, T], F32, name="rc8")

            vA = [v_p.tile([P, DA * H], BF16, name=f"vA{t}") for t in range(NT)]
            yTp = [yT_p.tile([P, T], BF16, name=f"yTp{cb}") for cb in range(NCB)]
            wpB = [wpB_p.tile([P, C], BF16, name=f"wpB{cb}") for cb in range(NCB)]
            for tb in range(NT):
                nc.vector.tensor_copy(vA[tb][:, D::DA], onesHB)

            wB_p = tc.alloc_tile_pool(name="wbf", bufs=1)
            wvB = [wB_p.tile([P, C], BF16, name=f"wvB{cb}") for cb in range(NCB)]
            wqkB = [wB_p.tile([P, C2], BF16, name=f"wqkB{cb}") for cb in range(NCB)]
            xT_p = tc.alloc_tile_pool(name="xT", bufs=1)
            xT = [xT_p.tile([P, T], BF16, name=f"xT{cb}") for cb in range(NCB)]
            wst_p = tc.alloc_tile_pool(name="wst", bufs=2)
            x_p = tc.alloc_tile_pool(name="xl", bufs=4)

            # ---- input DMAs: x on the Sync queue, weights on Scalar ----
            x_t = []
            for tb in range(4):
                xt = x_p.tile([P, C], F32R, name="x_t", tag="xt")
                nc.sync.dma_start(xt[:], x_d[tb * P:(tb + 1) * P, :].bitcast(F32R))
                x_t.append(xt)
            nc.scalar.dma_start(
                bvb[:],
                ba_d.unsqueeze(0)[:, C2:C3].partition_broadcast(P).squeeze(1))
            for cb in range(NCB):
                w = wst_p.tile([P, C], F32, name="wvS", tag="wst3")
                nc.scalar.dma_start(w[:], wa_d[cb * P:(cb + 1) * P, C2:C3])
                nc.scalar.activation(wvB[cb][:], w[:], Copy)
            nc.scalar.dma_start(bqk[:],
                                ba_d.rearrange("(a b) -> b a", b=P)[:, 0:NQK])

            # ---- x^T transposes (fp32r), copyback casts to bf16 ----
            for jt in range(2):
                for cb in range(NCB):
                    tr_ps = ps.tile([P, 512], F32R, name="tr_ps", tag="bank1",
                                    bufs=2)
                    for k in range(4):
                        nc.tensor.transpose(
                            tr_ps[:, k * P:(k + 1) * P],
                            x_t[4 * jt + k][:, cb * P:(cb + 1) * P],
                            identR)
                    nc.vector.tensor_copy(xT[cb][:, jt * 512:(jt + 1) * 512],
                                          tr_ps.bitcast(F32)[:])
                if jt == 0:
                    for tb in range(4, NT):
                        xt = x_p.tile([P, C], F32R, name="x_t", tag="xt")
                        nc.sync.dma_start(xt[:],
                                          x_d[tb * P:(tb + 1) * P, :].bitcast(F32R))
                        x_t.append(xt)
                    for cb in range(NCB):
                        w = wst_p.tile([P, C2], F32, name="wqkS", tag="wst6")
                        nc.scalar.dma_start(w[:], wa_d[cb * P:(cb + 1) * P, 0:C2])
                        nc.scalar.activation(wqkB[cb][:], w[:], Copy)
            x_p.release()

            # ---- v projection (bf16), 3D-AP copyback packs v_aug ----
            for tb in range(NT):
                v_ps = ps.tile([P, C], F32, name="v_ps", tag="big", bufs=3)
                for cb in range(NCB):
                    lhsT = xT[cb][:, tb * P:(tb + 1) * P]
                    nc.tensor.matmul(v_ps[:, 0:512], lhsT, wvB[cb][:, 0:512],
                                     start=(cb == 0), stop=(cb == NCB - 1))
                    nc.tensor.matmul(v_ps[:, 512:C], lhsT, wvB[cb][:, 512:C],
                                     start=(cb == 0), stop=(cb == NCB - 1))
                nc.vector.tensor_add(
                    vA[tb].rearrange("p (h e) -> p h e", e=DA)[:, :, 0:D],
                    v_ps.rearrange("p (h e) -> p h e", e=D),
                    bvb.rearrange("p (h e) -> p h e", e=D))

            # W_proj staging DMAs go out early; casts happen much later in the
            # ACT gap after exp5 (the engine is exp-saturated during heads)
            nc.scalar.dma_start(
                bpb[:], bp_d.unsqueeze(0).partition_broadcast(P).squeeze(1))
            wpS = []
            for cb in range(NCB):
                w = wst_p.tile([P, C], F32, name="wpS", tag="wps3", bufs=6)
                nc.scalar.dma_start(w[:], wp_d[cb * P:(cb + 1) * P, :])
                wpS.append(w)

            # ---------------- dense qk proj + per-head attention ----------------
            qkTd = [None] * NQK
            qTh = [None] * H
            kTh = [None] * H
            ptiles = [None] * H
            y_sbl = [None] * H
            y_sbr = [None] * H
            o_acc = [None] * NT

            def dense_half(m, half):
                # one 512-token half of dense projection block m: 6 matmuls
                # (~1.3us PE) + bias-add copyback casting to bf16
                def emit():
                    if qkTd[m] is None:
                        qkTd[m] = qkd_p.tile([P, T], BF16, name=f"qkTd{m}",
                                             tag="qkTd")
                    sl = slice(half * 512, (half + 1) * 512)
                    qk_ps = ps.tile([P, 512], F32, name="qk_ps", tag="bank1",
                                    bufs=2)
                    for cb in range(NCB):
                        nc.tensor.matmul(qk_ps[:], wqkB[cb][:, m * P:(m + 1) * P],
                                         xT[cb][:, sl],
                                         start=(cb == 0), stop=(cb == NCB - 1))
                    nc.vector.tensor_scalar_add(qkTd[m][:, sl], qk_ps[:],
                                                bqk[:, m:m + 1])
                return emit

            def dense_pair_units(j):
                return [dense_half(j, 0), dense_half(j, 1),
                        dense_half(NCB + j, 0), dense_half(NCB + j, 1)]

            def emit_repack(h):
                q = qkh_p.tile([D, T], BF16, name=f"qTh{h}", tag="qkh")
                k = qkh_p.tile([D, T], BF16, name=f"kTh{h}", tag="qkh")
                for dst, moff in ((q, 0), (k, NCB)):
                    for (m, r, a, ln) in head_segs(h):
                        nc.sync.dma_start(dst[a:a + ln, :],
                                          qkTd[moff + m][r:r + ln, :])
                qTh[h], kTh[h] = q, k

            def emit_S(h, fillers=()):
                fillers = list(fillers)
                pt = []
                for ib in range(NT):
                    q0 = P * ib
                    s_ps = ps.tile([P, T], F32, name="s_ps", tag="big", bufs=3)
                    kblk = kTh[h][:, ib * P:(ib + 1) * P]
                    if q0 < 512:
                        nc.tensor.matmul(s_ps[:, q0:512], kblk,
                                         qTh[h][:, q0:512], start=True, stop=True)
                    r0 = max(q0, 512)
                    nc.tensor.matmul(s_ps[:, r0:T], kblk,
                                     qTh[h][:, r0:T], start=True, stop=True)
                    p_t = p_p.tile([P, T], BF16, name="p_t", tag="pt")
                    nc.scalar.activation(p_t[:, q0:T], s_ps[:, q0:T],
                                         Exp, scale=SCALE)
                    # zero the upper triangle of the diagonal 128-col block
                    nc.vector.tensor_mul(p_t[:, q0:q0 + P],
                                         p_t[:, q0:q0 + P], triB)
                    pt.append(p_t)
                    if ib in (2, 4, 6, 7) and fillers:
                        fillers.pop(0)()
                for f in fillers:
                    f()
                ptiles[h] = pt

            def emit_PV(h):
                y_l = ps.tile([DA, 512], F32, name="y_l", tag="bank1", bufs=2)
                y_r = ps.tile([DA, 512], F32, name="y_r", tag="bank1", bufs=2)
                pt = ptiles[h]
                for ib in range(NT):
                    q0 = P * ib
                    va = vA[ib][:, DA * h:DA * h + DA]
                    if q0 < 512:
                        nc.tensor.matmul(y_l[:, q0:512], va, pt[ib][:, q0:512],
                                         start=(ib == 0), stop=(ib == 3))
                        nc.tensor.matmul(y_r[:], va, pt[ib][:, 512:T],
                                         start=(ib == 0), stop=False)
                    else:
                        nc.tensor.matmul(y_r[:, q0 - 512:512], va,
                                         pt[ib][:, q0:T],
                                         start=False, stop=(ib == NT - 1))
                ptiles[h] = None
                ysl = ysb_p.tile([DA, 512], BF16, name=f"ysl{h}", tag="ysb")
                ysr = ysb_p.tile([DA, 512], BF16, name=f"ysr{h}", tag="ysb")
                nc.vector.tensor_copy(ysl[:], y_l[:])
                nc.vector.tensor_copy(ysr[:], y_r[:])
                # gather the denominator rows onto partition h (Sync queue)
                nc.sync.dma_start(den8[h:h + 1, 0:512], ysl[D:DA, :])
                nc.sync.dma_start(den8[h:h + 1, 512:T], ysr[D:DA, :])
                y_sbl[h], y_sbr[h] = ysl, ysr

            def emit_recip(h0, h1):
                nc.vector.reciprocal(rc8[h0:h1, :], den8[h0:h1, :])

            def emit_tail(h):
                # rank-1 PE broadcast of the reciprocal row across partitions
                # (PE operands must start at partition 0: hop the row down
                # via a small SBUF->SBUF DMA first)
                rr = rcr_p.tile([1, T], F32R, name="rcr", tag="rcr")
                nc.sync.dma_start(rr[:], rc8.bitcast(F32R)[h:h + 1, :])
                bc_ps = ps.tile([D, T], F32, name="bc_ps", tag="big", bufs=3)
                nc.tensor.matmul(bc_ps[:, 0:512], onesMR[:],
                                 rr[:, 0:512], start=True, stop=True)
                nc.tensor.matmul(bc_ps[:, 512:T], onesMR[:],
                                 rr[:, 512:T], start=True, stop=True)
                yn_l = yn_p.tile([D, 512], BF16, name="yn_l", tag="yn")
                yn_r = yn_p.tile([D, 512], BF16, name="yn_r", tag="yn")
                nc.vector.tensor_mul(yn_l[:], y_sbl[h][0:D, :], bc_ps[:, 0:512])
                nc.vector.tensor_mul(yn_r[:], y_sbr[h][0:D, :], bc_ps[:, 512:T])
                for src, qsl in ((yn_l, slice(0, 512)), (yn_r, slice(512, T))):
                    for (m, r, a, ln) in head_segs(h):
                        nc.sync.dma_start(yTp[m][r:r + ln, qsl], src[a:a + ln, :])

            def G1_unit(tb):
                # out-proj contribution of feature blocks cb0..3: fully
                # written once heads 0..5 have scattered (head 5's first
                # segment ends cb3; head 6 starts at cb4[64:])
                def emit():
                    o_ps = ps.tile([P, C], F32, name="o_ps", tag="big", bufs=3)
                    for sl in (slice(0, 512), slice(512, C)):
                        for cb in range(4):
                            nc.tensor.matmul(o_ps[:, sl],
                                             yTp[cb][:, tb * P:(tb + 1) * P],
                                             wpB[cb][:, sl],
                                             start=(cb == 0), stop=(cb == 3))
                    for sl in (slice(0, 512), slice(512, C)):
                        nc.vector.tensor_add(o_acc[tb][:, sl], o_ps[:, sl],
                                             bpb[:, sl])
                return emit

            def emit_G2(tb):
                o_ps = ps.tile([P, C], F32, name="o_ps2", tag="big", bufs=3)
                for sl in (slice(0, 512), slice(512, C)):
                    for cb in (4, 5):
                        nc.tensor.matmul(o_ps[:, sl],
                                         yTp[cb][:, tb * P:(tb + 1) * P],
                                         wpB[cb][:, sl],
                                         start=(cb == 4), stop=(cb == 5))
                for sl in (slice(0, 512), slice(512, C)):
                    nc.vector.tensor_add(o_acc[tb][:, sl], o_acc[tb][:, sl],
                                         o_ps[:, sl])
                    nc.scalar.dma_start(out_d[tb * P:(tb + 1) * P, sl],
                                        o_acc[tb][:, sl])

            # ---- software-pipelined emission ----
            for u in dense_pair_units(0):
                u()
            emit_repack(0)
            emit_S(0, dense_pair_units(1))
            emit_repack(1)
            emit_PV(0)
            emit_S(1, dense_pair_units(2))
            emit_repack(2); emit_repack(3)
            emit_PV(1)
            emit_S(2, dense_pair_units(3))
            emit_repack(4)
            emit_PV(2)
            emit_S(3, dense_pair_units(4))
            emit_repack(5)
            emit_PV(3)
            emit_recip(0, 4)
            for h in range(4):
                emit_tail(h)
            p5 = dense_pair_units(5)
            emit_S(4, p5[0:2])
            emit_PV(4)
            emit_S(5, p5[2:4])
            emit_repack(6); emit_repack(7)
            emit_PV(5)
            emit_recip(0, 6)
            emit_tail(4); emit_tail(5)
            # W_proj casts land in the ACT gap between exp5 and exp6
            for cb in range(NCB):
                nc.scalar.activation(wpB[cb][:], wpS[cb][:], Copy)
            # dense projection inputs are dead; reuse the space for o_acc
            wst_p.release()
            xT_p.release()
            wB_p.release()
            oacc_p = tc.alloc_tile_pool(name="oacc", bufs=1)
            for tb in range(NT):
                o_acc[tb] = oacc_p.tile([P, C], F32, name=f"oacc{tb}")
            emit_S(6, [G1_unit(tb) for tb in range(0, 4)])
            emit_PV(6)
            emit_recip(0, 7)
            emit_tail(6)
            emit_S(7, [G1_unit(tb) for tb in range(4, NT)])
            emit_PV(7)
            emit_recip(0, 8)
            emit_tail(7)
            for tb in range(NT):
                emit_G2(tb)
            oacc_p.release()

    nc.compile()
    return nc


def run(inputs, trace=False):
    import concourse.bass_utils as bass_utils

    nc = _CACHE.get("nc")
    if nc is None:
        nc = _CACHE["nc"] = _build()

    x = np.ascontiguousarray(inputs["x"], dtype=np.float32)
    wa = np.ascontiguousarray(inputs["W_attn"], dtype=np.float32)
    ba = np.ascontiguousarray(inputs["b_attn"], dtype=np.float32)
    wp = np.ascontiguousarray(inputs["W_proj"], dtype=np.float32)
    bp = np.ascontiguousarray(inputs["b_proj"], dtype=np.float32)
    B = x.shape[0]
    in_maps = [
        {"x": np.ascontiguousarray(x[b]), "W_attn": wa, "b_attn": ba,
         "W_proj": wp, "b_proj": bp}
        for b in range(B)
    ]
    res = bass_utils.run_bass_kernel_spmd(
        nc, in_maps, core_ids=list(range(B)), trace=trace)
    out = np.stack([r["out"] for r in res.results], axis=0)
    return out, res


def kernel(**inputs):
    out, _ = run(inputs, trace=False)
    return out
